# revision 1
# baseline (speedup 1.0000x reference)
"""Causal self-attention (dense transformer block) on 8 Trainium2 NeuronCores.

Problem: B=2, T=2048, C=1024, H=16 heads, D=64.
    qkv = x @ w_attn + b_attn ; causal softmax attention ; out = y @ w_proj + b_proj

Sharding: 2-way data parallel on batch x 4-way tensor parallel on heads
(4 heads per core). c_attn is column-parallel, c_proj row-parallel; the
row-parallel all-reduce (sum of 4 partials per batch) + b_proj add happen on
the host at gather time.

Per-core kernel dataflow (zero on-device transposes):
  - x.T is augmented with a ones row so QKV biases fold into the matmuls
    (Q/K biases instead ride the PSUM->SBUF copy as per-partition ACT bias).
  - Q and K are computed directly transposed: Qt/Kt [d, T] = W.T @ x.T.
  - V is computed token-major with an extra ones column per head, so the
    P@V matmul also accumulates the softmax denominator l as output row 64.
  - S tiles are computed k-major ([k, q]), exp'd on ScalarE (softmax without
    max-subtraction: |scores| <= ~4 here, exactly softmax-equivalent), the
    diagonal tiles causal-masked by multiplication on VectorE (fully-masked
    128-wide strips are skipped entirely), and fed straight into P@V.
  - Attention is software-pipelined across (head, q-chunk) units: unit i's
    P@V matmuls interleave with unit i+1's S matmuls so ScalarE exp always
    overlaps PE work; each 512-token slab is projected + stored as soon as
    its 4 heads finish.
  - 1/l is broadcast across partitions on GPSIMD and applied on VectorE,
    leaving y head-dim-major [hd, T] - exactly the lhsT the projection needs.
  - All matmuls run as float32r (full-rate fp32 path; measured end-to-end
    scale-relative error vs fp32 reference: 2.2e-4).
"""

from contextlib import ExitStack

import numpy as np

import concourse.bacc as bacc
import concourse.bass as bass
import concourse.mybir as mybir
import concourse.tile as tile
from concourse import bass_utils

B, T, C, H = 2, 2048, 1024, 16
D = C // H
NH = 4                      # heads per core
N_CORES = 8
P = 128
CH = (C + 1 + P - 1) // P   # 9 contraction chunks (x.T + ones row, padded)
VW = NH * (D + 1)           # V width incl. per-head ones columns
F32 = mybir.dt.float32
F32R = mybir.dt.float32r
USE_F32R = True


def build_tile_kernel(tc, ins, out, T=2048, C=1024, NH=4, use_f32r=True):
    """ins: dict name->AP. out: AP [T, C]."""
    nc = tc.nc
    D = 64
    CH = (C + 1 + P - 1) // P  # 9 c-chunks of 128 (x.T padded with ones+zeros)
    NP = NH // 2               # head pairs
    NTC = T // 512             # t-chunks (Qt/Kt free dim + query chunks)
    NTT = T // P               # token tiles
    NCC = C // 512             # proj output column chunks
    VW = NH * (D + 1)          # V width incl. ones cols (260)
    scale = 1.0 / np.sqrt(D)

    MDT = F32R if use_f32r else F32

    def mm(out_ap, lhsT, rhs, **kw):
        nc.tensor.matmul(out_ap, lhsT, rhs, **kw)

    xta = ins["xta"]      # [CH*128, T]
    wqa = ins["wqa"]      # [CH*128, NH*64]
    wka = ins["wka"]      # [CH*128, NH*64]
    wva = ins["wva"]      # [CH*128, VW]
    wp = ins["wp"]        # [NH*64, C]
    msk = ins["msk"]      # [128, 4*512] diagonal causal masks

    with ExitStack() as stk:
        const_pool = stk.enter_context(tc.tile_pool(name="const", bufs=1))
        qkv_sb = stk.enter_context(tc.tile_pool(name="qkv_sb", bufs=1))

        # --- load weights/masks ---
        wp_sb = const_pool.tile([P, 2, C], MDT, tag="wp")
        nc.scalar.dma_start(wp_sb[:], wp.rearrange("(c p) n -> p c n", p=P))
        msk_sb = const_pool.tile([P, 4, 512], MDT, tag="msk")
        nc.scalar.dma_start(msk_sb[:], msk.rearrange("p (j f) -> p j f", j=4))
        bqk_sb = const_pool.tile([P, 2 * NP], F32, tag="bqk")
        nc.scalar.dma_start(bqk_sb[:], ins["bqk"][:, :])

        # --- persistent activations ---
        qt_all = qkv_sb.tile([P, NP, T], MDT, tag="qt")   # [pair*(2x64), pair, T]
        kt_all = qkv_sb.tile([P, NP, T], MDT, tag="kt")
        v_all = qkv_sb.tile([P, NTT, VW], MDT, tag="v")
        yt_all = qkv_sb.tile([P, NP, T], MDT, tag="yt")

        # ---------------- phase 1: QKV projections ----------------
        with ExitStack() as s1:
            xpool = s1.enter_context(tc.tile_pool(name="xchunk", bufs=2))
            w1_pool = s1.enter_context(tc.tile_pool(name="w1", bufs=1))
            wqa_sb = w1_pool.tile([P, CH, NH * D], MDT, tag="wqa")
            nc.scalar.dma_start(wqa_sb[:], wqa.rearrange("(c p) n -> p c n", p=P))
            wka_sb = w1_pool.tile([P, CH, NH * D], MDT, tag="wka")
            nc.scalar.dma_start(wka_sb[:], wka.rearrange("(c p) n -> p c n", p=P))
            wva_sb = w1_pool.tile([P, CH, VW], MDT, tag="wva")
            nc.scalar.dma_start(wva_sb[:], wva.rearrange("(c p) n -> p c n", p=P))
            qk_ps = s1.enter_context(
                tc.tile_pool(name="qk_ps", bufs=3, space="PSUM")
            )
            v_ps = s1.enter_context(tc.tile_pool(name="v_ps", bufs=2, space="PSUM"))
            for ti in range(NTC):
                xc = xpool.tile([P, CH - 1, 512], MDT, tag="xc")
                xta_r = xta.rearrange("(c p) t -> p c t", p=P)
                nc.sync.dma_start(
                    xc[:, 0:4, :], xta_r[:, 0:4, bass.ts(ti, 512)]
                )
                nc.scalar.dma_start(
                    xc[:, 4 : CH - 1, :], xta_r[:, 4 : CH - 1, bass.ts(ti, 512)]
                )
                for j in range(NP):
                    for qk, (w_sb, dst) in enumerate(
                        ((wqa_sb, qt_all), (wka_sb, kt_all))
                    ):
                        ps = qk_ps.tile([P, 512], F32, tag="qk")
                        for c in range(CH - 1):
                            mm(
                                ps[:],
                                w_sb[:, c, bass.ts(j, P)],
                                xc[:, c, :],
                                start=(c == 0),
                                stop=(c == CH - 2),
                            )
                        nc.scalar.activation(
                            dst[:, j, bass.ts(ti, 512)], ps[:],
                            mybir.ActivationFunctionType.Identity,
                            bias=bqk_sb[:, qk * NP + j : qk * NP + j + 1],
                        )
                for tt in range(4):
                    ps = v_ps.tile([P, VW], F32, tag="v")
                    for c in range(CH - 1):
                        mm(
                            ps[:],
                            xc[:, c, bass.ts(tt, P)],
                            wva_sb[:, c, :],
                            start=(c == 0),
                            stop=False,
                        )
                    mm(
                        ps[:],
                        msk_sb[0:1, 0, 0:P],
                        wva_sb[0:1, CH - 1, :],
                        start=False,
                        stop=True,
                    )
                    nc.scalar.copy(v_all[:, ti * 4 + tt, :], ps[:])

        # ---- phase 2+3: attention + interleaved proj, software-pipelined
        # across (h, qc) units: unit i's P@V matmuls interleave with unit
        # i+1's S matmuls so ScalarE exp always overlaps PE work.
        with ExitStack() as s2:
            st_ps = s2.enter_context(tc.tile_pool(name="st_ps", bufs=4, space="PSUM"))
            ot_ps = s2.enter_context(tc.tile_pool(name="ot_ps", bufs=2, space="PSUM"))
            pj_ps = s2.enter_context(tc.tile_pool(name="pj_ps", bufs=2, space="PSUM"))
            pt_pool = s2.enter_context(tc.tile_pool(name="pt", bufs=33))
            nrm_pool = s2.enter_context(tc.tile_pool(name="nrm", bufs=2))
            ostage = s2.enter_context(tc.tile_pool(name="ostage", bufs=4))

            def emit_pv(u):
                """Emit one pending P@V matmul of unit u; True if one was left."""
                if not u or not u["pvs"]:
                    return False
                kt, pt, lo = u["pvs"].pop(0)
                if u["ot"] is None:
                    ot_tile = ot_ps.tile([D + 1, 512], F32, tag="ot")
                    u["ot"] = ot_tile
                mm(
                    u["ot"][:, lo:512],
                    v_all[:, kt, u["h"] * (D + 1) : (u["h"] + 1) * (D + 1)],
                    pt[:, lo:512],
                    start=(kt == 0),
                    stop=(kt == u["nkt"] - 1),
                )
                return True

            def finish_unit(u):
                """Normalize y for unit u; emit proj for its slab when last head."""
                h, qc, hb, hj = u["h"], u["qc"], (u["h"] % 2) * D, u["h"] // 2
                recip = nrm_pool.tile([1, 512], F32, tag="recip")
                nc.vector.reciprocal(recip[:], u["ot"][D : D + 1, :])
                rb = nrm_pool.tile([D, 512], F32, tag="rb")
                nc.gpsimd.partition_broadcast(rb[:], recip[:])
                nc.vector.tensor_mul(
                    yt_all[hb : hb + D, hj, bass.ts(qc, 512)],
                    u["ot"][0:D, :],
                    rb[:],
                )
                if h == NH - 1:
                    for tt in range(qc * 4, qc * 4 + 4):
                        for cc in range(NCC):
                            ps = pj_ps.tile([P, 512], F32, tag="pj")
                            for j in range(NP):
                                mm(
                                    ps[:],
                                    yt_all[:, j, bass.ts(tt, P)],
                                    wp_sb[:, j, bass.ts(cc, 512)],
                                    start=(j == 0),
                                    stop=(j == NP - 1),
                                )
                            st = ostage.tile([P, 512], F32, tag="os")
                            nc.vector.tensor_copy(st[:], ps[:])
                            nc.sync.dma_start(
                                out[bass.ts(tt, P), bass.ts(cc, 512)], st[:]
                            )

            live = []

            def pump():
                if live and not live[0]["pvs"]:
                    finish_unit(live.pop(0))
                if live:
                    emit_pv(live[0])

            for qc in range(NTC):
                for h in range(NH):
                    hb, hj = (h % 2) * D, h // 2
                    nkt = 4 * (qc + 1)
                    cur = {"h": h, "qc": qc, "nkt": nkt, "pvs": [], "ot": None}
                    for kt in range(nkt):
                        pump()
                        j = kt - (nkt - 4)
                        lo = max(j, 0) * P  # skip fully-masked strips
                        st = st_ps.tile([P, 512], F32, tag="st")
                        mm(
                            st[:, lo:512],
                            kt_all[hb : hb + D, hj, bass.ts(kt, P)],
                            qt_all[hb : hb + D, hj, qc * 512 + lo : qc * 512 + 512],
                            start=True,
                            stop=True,
                        )
                        pt = pt_pool.tile([P, 512], MDT, tag="pt")
                        nc.scalar.activation(
                            pt[:, lo:512], st[:, lo:512],
                            mybir.ActivationFunctionType.Exp,
                            scale=float(scale),
                        )
                        if j >= 0:
                            nc.vector.tensor_mul(
                                pt[:, lo:512], pt[:, lo:512], msk_sb[:, j, lo:512]
                            )
                        cur["pvs"].append((kt, pt, lo))
                    live.append(cur)
                    lag = 2 if qc < 2 else 1
                    while len(live) > lag:
                        u = live.pop(0)
                        while emit_pv(u):
                            pass
                        finish_unit(u)
            while live:
                u = live.pop(0)
                while emit_pv(u):
                    pass
                finish_unit(u)



def make_shard_inputs(x_b, w_attn, b_attn, w_proj, h0, NH=4, T=2048, C=1024):
    """Per-core input dict for batch slice x_b [T, C] and heads h0..h0+NH-1."""
    D = 64
    CH = (C + 1 + P - 1) // P
    VW = NH * (D + 1)
    xta = np.zeros((CH * P, T), dtype=np.float32)
    xta[:C] = x_b.T
    xta[C] = 1.0

    def aug(w_cols, b_cols):
        wa = np.zeros((CH * P, w_cols.shape[1]), dtype=np.float32)
        wa[:C] = w_cols
        wa[C] = b_cols
        return wa

    qs = slice(h0 * D, (h0 + NH) * D)
    ks = slice(C + h0 * D, C + (h0 + NH) * D)
    wqa = aug(w_attn[:, qs], b_attn[qs])
    wka = aug(w_attn[:, ks], b_attn[ks])
    wva = np.zeros((CH * P, VW), dtype=np.float32)
    for h in range(NH):
        vs = slice(2 * C + (h0 + h) * D, 2 * C + (h0 + h + 1) * D)
        wva[:C, h * (D + 1) : h * (D + 1) + D] = w_attn[:, vs]
        wva[C, h * (D + 1) : h * (D + 1) + D] = b_attn[vs]
        wva[C, h * (D + 1) + D] = 1.0  # ones column -> softmax denominator

    wp = np.ascontiguousarray(
        w_proj[h0 * D : (h0 + NH) * D, :], dtype=np.float32
    )

    msk = np.zeros((P, 4 * 512), dtype=np.float32)
    for j in range(4):
        p = np.arange(P)[:, None]
        f = np.arange(512)[None, :]
        msk[:, j * 512 : (j + 1) * 512] = (j * P + p <= f).astype(np.float32)

    bqk = np.zeros((P, 4), dtype=np.float32)
    NP = NH // 2
    for j in range(NP):
        bqk[:, j] = b_attn[(h0 + 2 * j) * D : (h0 + 2 * j + 2) * D]
        bqk[:, NP + j] = b_attn[C + (h0 + 2 * j) * D : C + (h0 + 2 * j + 2) * D]

    return {
        "xta": xta,
        "wqa": wqa,
        "wka": wka,
        "wva": wva,
        "wp": wp,
        "msk": msk,
        "bqk": bqk,
    }




_NC_CACHE = {}


def _build_nc():
    if "nc" in _NC_CACHE:
        return _NC_CACHE["nc"]
    nc = bacc.Bacc("TRN2", target_bir_lowering=False, debug=False)
    in_specs = {
        "xta": (CH * P, T),
        "wqa": (CH * P, NH * D),
        "wka": (CH * P, NH * D),
        "wva": (CH * P, VW),
        "wp": (NH * D, C),
        "msk": (P, 4 * 512),
        "bqk": (P, 4),
    }
    mdt = F32R if USE_F32R else F32
    in_aps = {
        k: nc.dram_tensor(
            k, list(s), F32 if k == "bqk" else mdt, kind="ExternalInput"
        ).ap()
        for k, s in in_specs.items()
    }
    out_ap = nc.dram_tensor("out", [T, C], F32, kind="ExternalOutput").ap()
    with tile.TileContext(nc) as tc:
        build_tile_kernel(tc, in_aps, out_ap, T=T, C=C, NH=NH, use_f32r=USE_F32R)
    nc.compile()
    _NC_CACHE["nc"] = nc
    return nc


def _run(inputs, trace=False):
    x = np.ascontiguousarray(inputs["x"], dtype=np.float32)
    w_attn = np.ascontiguousarray(inputs["w_attn"], dtype=np.float32)
    b_attn = np.ascontiguousarray(inputs["b_attn"], dtype=np.float32)
    w_proj = np.ascontiguousarray(inputs["w_proj"], dtype=np.float32)
    b_proj = np.ascontiguousarray(inputs["b_proj"], dtype=np.float32)

    nc = _build_nc()
    in_maps = [
        make_shard_inputs(
            x[c // 4], w_attn, b_attn, w_proj, (c % 4) * NH, NH=NH, T=T, C=C
        )
        for c in range(N_CORES)
    ]
    res = bass_utils.run_bass_kernel_spmd(
        nc, in_maps, core_ids=list(range(N_CORES)), trace=trace
    )
    out = np.zeros((B, T, C), dtype=np.float64)
    for c in range(N_CORES):
        out[c // 4] += res.results[c]["out"].astype(np.float64)
    out += b_proj.astype(np.float64)
    return out.astype(np.float32), res


def kernel(**inputs):
    out, _ = _run(inputs)
    return out



# revision 10
# speedup vs baseline: 1.1530x; 1.1530x over previous
"""Causal self-attention (dense transformer block) on 8 Trainium2 NeuronCores.

Problem: B=2, T=2048, C=1024, H=16 heads, D=64.
    qkv = x @ w_attn + b_attn ; causal softmax attention ; out = y @ w_proj + b_proj

Sharding: 2-way data parallel on batch x 4-way tensor parallel on heads
(4 heads per core). c_attn is column-parallel, c_proj row-parallel; the
row-parallel all-reduce (sum of 4 partials per batch) + b_proj add happen on
the host at gather time.

v2 design (single fused pipeline, engine-balanced):
  - 5 rounds r=0..4. Round r computes QKV for token chunk ti=r (r<=3) and
    runs attention units (h, qc) for qc=r-1 interleaved, so ScalarE exp
    always overlaps PE work across the whole kernel.
  - x and QKV weights are bf16 (halves input DMA); Q/K kept f32(r) in SBUF,
    scores computed in f32r; P, V, y are bf16.
  - PV is computed transposed: ot[q,(d|l)] += ptT[k,q].T @ v[k,(d|l)] with a
    bf16 moving operand (65 cols @ 1 cycle/row) - half the PE columns of the
    [d,q]-major form. Per-head ones-column in V accumulates the softmax
    denominator l.
  - Normalization is per-PARTITION (1/l via DVE reciprocal + GPSIMD
    tensor_scalar), then a DMA-engine XBAR transpose (dma_start_transpose)
    produces the head-major y.T layout the projection needs: no PE or
    partition-broadcast cost.
  - Engine placement: PE matmuls only; Act exp only; DVE = QK bias-add,
    causal mask (bf16 4x mode), reciprocals, proj PSUM->SBUF staging;
    V copies + normalize also DVE (GPSIMD cannot touch PSUM); SP issues all DMAs.
  - PSUM: qk ring(2) + shared V/proj ring(2) + score ring(2) + ot(2) = 8;
    each PV qt-group is contiguous per PSUM bank (one open accum group/bank).
"""

from contextlib import ExitStack

import ml_dtypes
import numpy as np

import concourse.bacc as bacc
import concourse.bass as bass
import concourse.mybir as mybir
import concourse.tile as tile
from concourse import bass_utils

B, T, C, H = 2, 2048, 1024, 16
D = 64
NH = 4                      # heads per core
NP = NH // 2                # head pairs
N_CORES = 8
P = 128
CHX = C // P                # 8 x/w contraction chunks
NTC = T // 512              # 4 t-chunks
VW = NH * (D + 1)           # 260: V width incl. per-head ones column
F32 = mybir.dt.float32
F32R = mybir.dt.float32r
BF16 = mybir.dt.bfloat16

# round -> list of (head, q-chunk) attention units
SCHED = {
    0: [],
    1: [(h, 0) for h in range(NH)],
    2: [(h, 1) for h in range(NH)],
    3: [(h, 2) for h in range(NH)],
    4: [(h, 3) for h in range(NH)],
}


def build_tile_kernel(tc, ins, out):
    nc = tc.nc
    scale = 1.0 / np.sqrt(D)

    xtb = ins["xtb"]    # [C, T] bf16 (x_b.T)
    wqa = ins["wqa"]    # [C, NH*D] bf16
    wka = ins["wka"]    # [C, NH*D] bf16
    wva = ins["wva"]    # [(CHX+1)*P, VW] bf16 (row C = bias / ones-col)
    wp = ins["wp"]      # [NH*D, C] bf16
    msk = ins["msk"]    # [P, 4*512] bf16 diagonal causal masks (row0 j0 = ones)
    bqk = ins["bqk"]    # [P, 4] f32 per-pair Q/K biases

    with ExitStack() as stk:
        pool = lambda name, bufs, space="SBUF": stk.enter_context(
            tc.tile_pool(name=name, bufs=bufs, space=space)
        )
        const = pool("const", 1)
        xpool = pool("xc", 2)
        qkv_sb = pool("qkv", 1)
        pt_pool = pool("pt", 26)
        stg_pool = pool("stg", 8)
        rc_pool = pool("rc", 8)
        ost_pool = pool("ost", 4)
        qk_ps = pool("qk_ps", 2, "PSUM")
        big_ps = pool("big_ps", 2, "PSUM")   # shared: V chains + projection
        st_ps = pool("st_ps", 2, "PSUM")
        ot_ps = pool("ot_ps", 2, "PSUM")

        # --- constant loads, all on the SP (sync) HWDGE queue, ordered by
        # first use: bqk+wqa before x chunk 0, wp (projection) last. ---
        bqk_sb = const.tile([P, 4], F32, tag="bqk")
        nc.sync.dma_start(bqk_sb[:], bqk[:, :])
        wqa_sb = const.tile([P, CHX, NH * D], BF16, tag="wqa")
        nc.sync.dma_start(wqa_sb[:], wqa.rearrange("(c p) n -> p c n", p=P))

        # --- persistent activations ---
        qt_all = qkv_sb.tile([P, NP, T], F32R, tag="qt")  # [pair 2x64, pair, T]
        kt_all = qkv_sb.tile([P, NP, T], F32R, tag="kt")
        v_all = qkv_sb.tile([P, NTC * 4, VW], BF16, tag="v")   # [t, tt, d|l]
        yt_all = qkv_sb.tile([P, NP, T], BF16, tag="yt")  # [pair 2x64, pair, T]

        xr = xtb.rearrange("(c p) t -> p c t", p=P)
        xc_tiles = {}

        def issue_xc(r):
            xcr = xpool.tile([P, CHX, 512], BF16, tag="xc")
            nc.sync.dma_start(xcr[:, 0:4, :], xr[:, 0:4, bass.ts(r, 512)])
            nc.sync.dma_start(xcr[:, 4:8, :], xr[:, 4:8, bass.ts(r, 512)])
            xc_tiles[r] = xcr

        issue_xc(0)
        wka_sb = const.tile([P, CHX, NH * D], BF16, tag="wka")
        nc.sync.dma_start(wka_sb[:], wka.rearrange("(c p) n -> p c n", p=P))
        wva_sb = const.tile([P, CHX + 1, VW], BF16, tag="wva")
        nc.sync.dma_start(wva_sb[:], wva.rearrange("(c p) n -> p c n", p=P))
        msk_sb = const.tile([P, 4, 512], BF16, tag="msk")
        nc.sync.dma_start(msk_sb[:], msk.rearrange("p (j f) -> p j f", j=4))
        wp_sb = const.tile([P, NP, C], BF16, tag="wp")
        nc.sync.dma_start(wp_sb[:], wp.rearrange("(c p) n -> p c n", p=P))

        def emit_qk(r, which, jj):
            """Q (which=0) or K (which=1) chain for pair jj, t-chunk r."""
            w_sb, dst = ((wqa_sb, qt_all), (wka_sb, kt_all))[which]
            xcr = xc_tiles[r]
            ps = qk_ps.tile([P, 512], F32, tag="qk")
            for c in range(CHX):
                nc.tensor.matmul(
                    ps[:], w_sb[:, c, bass.ts(jj, P)], xcr[:, c, :],
                    start=(c == 0), stop=(c == CHX - 1),
                )
            nc.vector.tensor_scalar_add(
                dst[:, jj, bass.ts(r, 512)], ps[:],
                bqk_sb[:, which * NP + jj : which * NP + jj + 1],
            )

        def emit_v(r, tt):
            """V rows for token tile r*4+tt, all heads + ones cols."""
            xcr = xc_tiles[r]
            ps = big_ps.tile([P, 512], F32, tag="big")
            pv = ps[:, 0:VW]
            for c in range(CHX):
                nc.tensor.matmul(
                    pv, xcr[:, c, bass.ts(tt, P)], wva_sb[:, c, :],
                    start=(c == 0), stop=False,
                )
            # rank-1 bias/ones row via 1-partition matmul (msk row0/j0 is ones)
            nc.tensor.matmul(
                pv, msk_sb[0:1, 0, 0:P], wva_sb[0:1, CHX, :],
                start=False, stop=True,
            )
            nc.vector.tensor_copy(v_all[:, r * 4 + tt, :], pv)

        cur_stg = {}
        live = []  # units with pending PV qt-groups (PSUM: one open group/bank)

        def finish_unit(u):
            """Normalize by 1/l into pair staging; XBAR-transpose at pair end."""
            h, qc, ot = u["h"], u["qc"], u["ot"]
            hb, hj = (h % 2) * D, h // 2
            for qt in range(4):
                rc = rc_pool.tile([P, 1], F32, tag="rc")
                nc.vector.reciprocal(rc[:], ot[:, qt, D : D + 1])
                if h % 2 == 0:
                    cur_stg[(hj, qt)] = stg_pool.tile(
                        [P, P], BF16, tag="stg", name="stg"
                    )
                stg = cur_stg[(hj, qt)]
                nc.vector.tensor_scalar_mul(
                    stg[:, hb : hb + D], ot[:, qt, 0:D], rc[:]
                )
                if h % 2 == 1:
                    nc.sync.dma_start_transpose(
                        yt_all[:, hj, bass.ts(qc * 4 + qt, P)], stg[:]
                    )

        def pump_pv():
            """Emit the oldest unit's next PV qt-group (contiguous in ot bank)."""
            if not live or not live[0]["pvgs"]:
                return
            u = live[0]
            qt = u["pvgs"].pop(0)
            h, qc = u["h"], u["qc"]
            last = qc * 4 + qt
            for kt in range(last + 1):
                nc.tensor.matmul(
                    u["ot"][:, qt, :],
                    u["pts"][kt][:, bass.ts(qt, P)],
                    v_all[:, kt, h * (D + 1) : (h + 1) * (D + 1)],
                    start=(kt == 0), stop=(kt == last),
                )
            if not u["pvgs"]:
                live.pop(0)
                finish_unit(u)

        def emit_unit(h, qc):
            """S/exp/mask phase for unit (h, qc); PV of older units pumped in."""
            hb, hj = (h % 2) * D, h // 2
            nkt = 4 * (qc + 1)
            pts = []
            for kt in range(nkt):
                j = kt - (nkt - 4)
                lo = max(j, 0) * P
                st = st_ps.tile([P, 512], F32, tag="st")
                nc.tensor.matmul(
                    st[:, lo:512],
                    kt_all[hb : hb + D, hj, bass.ts(kt, P)],
                    qt_all[hb : hb + D, hj, qc * 512 + lo : (qc + 1) * 512],
                    start=True, stop=True,
                )
                pt = pt_pool.tile([P, 512], BF16, tag="pt")
                nc.scalar.activation(
                    pt[:, lo:512], st[:, lo:512],
                    mybir.ActivationFunctionType.Exp, scale=float(scale),
                )
                if j >= 0:
                    nc.vector.tensor_mul(
                        pt[:, lo:512], pt[:, lo:512], msk_sb[:, j, lo:512]
                    )
                pts.append(pt)
                pump_pv()
            ot = ot_ps.tile([P, 4, D + 1], F32, tag="ot")
            live.append(
                {"h": h, "qc": qc, "ot": ot, "pts": pts, "pvgs": [0, 1, 2, 3]}
            )

        def emit_proj(pq, g):
            """Projection group g (tt=pq*4+g//2, cc=g%2) for slab pq."""
            tt, cc = pq * 4 + g // 2, g % 2
            ps = big_ps.tile([P, 512], F32, tag="big")
            for jj in range(NP):
                nc.tensor.matmul(
                    ps[:], yt_all[:, jj, bass.ts(tt, P)],
                    wp_sb[:, jj, bass.ts(cc, 512)],
                    start=(jj == 0), stop=(jj == NP - 1),
                )
            st = ost_pool.tile([P, 512], F32, tag="ost")
            nc.vector.tensor_copy(st[:], ps[:])
            nc.sync.dma_start(out[bass.ts(tt, P), bass.ts(cc, 512)], st[:])

        # ---------------- the fused pipeline ----------------
        for r in range(5):
            units = SCHED[r]
            if r == 0:
                for jj in range(NP):
                    emit_qk(0, 0, jj)
                    emit_qk(0, 1, jj)
                issue_xc(1)
                for tt in range(4):
                    emit_v(0, tt)
                continue
            has_qkv = r <= 3
            if has_qkv:
                for jj in range(NP):
                    emit_qk(r, 0, jj)
                    emit_qk(r, 1, jj)
                if r + 1 <= 3:
                    issue_xc(r + 1)
                emit_v(r, 0)
                emit_v(r, 1)
            pq = r - 2  # projection slab ready from last round
            for i, (h, qc) in enumerate(units):
                emit_unit(h, qc)
                if has_qkv and i == 0:
                    emit_v(r, 2)
                    emit_v(r, 3)
                if pq >= 0:
                    for g in range(2 * i, min(2 * i + 2, 8)):
                        emit_proj(pq, g)
        while live:
            pump_pv()
        for g in range(8):
            emit_proj(3, g)


def make_shard_inputs(x_b, w_attn, b_attn, w_proj, h0):
    """Per-core input dict for batch slice x_b [T, C] and heads h0..h0+NH-1."""
    bf = ml_dtypes.bfloat16
    xtb = np.ascontiguousarray(x_b.T).astype(bf)

    qs = slice(h0 * D, (h0 + NH) * D)
    ks = slice(C + h0 * D, C + (h0 + NH) * D)
    wqa = np.ascontiguousarray(w_attn[:, qs]).astype(bf)
    wka = np.ascontiguousarray(w_attn[:, ks]).astype(bf)

    wva = np.zeros((C + P, VW), dtype=np.float32)
    for h in range(NH):
        vs = slice(2 * C + (h0 + h) * D, 2 * C + (h0 + h + 1) * D)
        wva[:C, h * (D + 1) : h * (D + 1) + D] = w_attn[:, vs]
        wva[C, h * (D + 1) : h * (D + 1) + D] = b_attn[vs]
        wva[C, h * (D + 1) + D] = 1.0  # ones column -> softmax denominator
    wva = wva.astype(bf)

    wp = np.ascontiguousarray(w_proj[h0 * D : (h0 + NH) * D, :]).astype(bf)

    msk = np.zeros((P, 4 * 512), dtype=np.float32)
    p = np.arange(P)[:, None]
    f = np.arange(512)[None, :]
    for j in range(4):
        msk[:, j * 512 : (j + 1) * 512] = (j * P + p <= f).astype(np.float32)
    msk = msk.astype(bf)

    bqk = np.zeros((P, 4), dtype=np.float32)
    for j in range(NP):
        bqk[:, j] = b_attn[(h0 + 2 * j) * D : (h0 + 2 * j + 2) * D]
        bqk[:, NP + j] = b_attn[C + (h0 + 2 * j) * D : C + (h0 + 2 * j + 2) * D]

    return {
        "xtb": xtb, "wqa": wqa, "wka": wka, "wva": wva,
        "wp": wp, "msk": msk, "bqk": bqk,
    }


_NC_CACHE = {}


def _build_nc():
    if "nc" in _NC_CACHE:
        return _NC_CACHE["nc"]
    nc = bacc.Bacc("TRN2", target_bir_lowering=False, debug=False)
    in_specs = {
        "xtb": ((C, T), BF16),
        "wqa": ((C, NH * D), BF16),
        "wka": ((C, NH * D), BF16),
        "wva": ((C + P, VW), BF16),
        "wp": ((NH * D, C), BF16),
        "msk": ((P, 4 * 512), BF16),
        "bqk": ((P, 4), F32),
    }
    in_aps = {
        k: nc.dram_tensor(k, list(s), dt, kind="ExternalInput").ap()
        for k, (s, dt) in in_specs.items()
    }
    out_ap = nc.dram_tensor("out", [T, C], F32, kind="ExternalOutput").ap()
    with tile.TileContext(nc) as tc:
        build_tile_kernel(tc, in_aps, out_ap)
    nc.compile()
    _NC_CACHE["nc"] = nc
    return nc


def _run(inputs, trace=False):
    x = np.ascontiguousarray(inputs["x"], dtype=np.float32)
    w_attn = np.ascontiguousarray(inputs["w_attn"], dtype=np.float32)
    b_attn = np.ascontiguousarray(inputs["b_attn"], dtype=np.float32)
    w_proj = np.ascontiguousarray(inputs["w_proj"], dtype=np.float32)
    b_proj = np.ascontiguousarray(inputs["b_proj"], dtype=np.float32)

    nc = _build_nc()
    in_maps = [
        make_shard_inputs(x[c // 4], w_attn, b_attn, w_proj, (c % 4) * NH)
        for c in range(N_CORES)
    ]
    res = bass_utils.run_bass_kernel_spmd(
        nc, in_maps, core_ids=list(range(N_CORES)), trace=trace
    )
    out = np.zeros((B, T, C), dtype=np.float64)
    for c in range(N_CORES):
        out[c // 4] += res.results[c]["out"].astype(np.float64)
    out += b_proj.astype(np.float64)
    return out.astype(np.float32), res


def kernel(**inputs):
    out, _ = _run(inputs)
    return out


# revision 11
# speedup vs baseline: 1.1734x; 1.0177x over previous
"""Causal self-attention (dense transformer block) on 8 Trainium2 NeuronCores.

Problem: B=2, T=2048, C=1024, H=16 heads, D=64.
    qkv = x @ w_attn + b_attn ; causal softmax attention ; out = y @ w_proj + b_proj

Sharding: 2-way data parallel on batch x 4-way tensor parallel on heads
(4 heads per core). c_attn is column-parallel, c_proj row-parallel; the
row-parallel all-reduce (sum of 4 partials per batch) + b_proj add happen on
the host at gather time.

v2 design (single fused pipeline, engine-balanced):
  - 5 rounds r=0..4. Round r computes QKV for token chunk ti=r (r<=3) and
    runs attention units (h, qc) for qc=r-1 interleaved, so ScalarE exp
    always overlaps PE work across the whole kernel.
  - x and QKV weights are bf16 (halves input DMA); Q/K kept f32(r) in SBUF,
    scores computed in f32r; P, V, y are bf16.
  - PV is computed transposed: ot[q,(d|l)] += ptT[k,q].T @ v[k,(d|l)] with a
    bf16 moving operand (65 cols @ 1 cycle/row) - half the PE columns of the
    [d,q]-major form. Per-head ones-column in V accumulates the softmax
    denominator l.
  - Normalization is per-PARTITION (1/l via DVE reciprocal + GPSIMD
    tensor_scalar), then a DMA-engine XBAR transpose (dma_start_transpose)
    produces the head-major y.T layout the projection needs: no PE or
    partition-broadcast cost.
  - Engine placement: PE matmuls only; Act exp only; DVE = QK bias-add,
    causal mask (bf16 4x mode), reciprocals, proj PSUM->SBUF staging;
    V copies + normalize also DVE (GPSIMD cannot touch PSUM); SP issues all DMAs.
  - PSUM: qk ring(2) + shared V/proj ring(2) + score ring(2) + ot(2) = 8;
    each PV qt-group is contiguous per PSUM bank (one open accum group/bank).
"""

from contextlib import ExitStack

import ml_dtypes
import numpy as np

import concourse.bacc as bacc
import concourse.bass as bass
import concourse.mybir as mybir
import concourse.tile as tile
from concourse import bass_utils

B, T, C, H = 2, 2048, 1024, 16
D = 64
NH = 4                      # heads per core
NP = NH // 2                # head pairs
N_CORES = 8
P = 128
CHX = C // P                # 8 x/w contraction chunks
NTC = T // 512              # 4 t-chunks
VW = NH * (D + 1)           # 260: V width incl. per-head ones column
F32 = mybir.dt.float32
F32R = mybir.dt.float32r
BF16 = mybir.dt.bfloat16

# round -> list of (head, q-chunk) attention units; unit (h, qc) is legal in
# any round >= qc (after that round's K chains). Spread so per-round Act(exp)
# stays below per-round PE work. Pairs (2j, 2j+1) stay together in order.
SCHED = {
    0: [],
    1: [(0, 0), (1, 0), (2, 0), (3, 0), (0, 1), (1, 1)],
    2: [(2, 1), (3, 1), (0, 2), (1, 2)],
    3: [(2, 2), (3, 2), (0, 3), (1, 3)],
    4: [(2, 3), (3, 3)],
}


def build_tile_kernel(tc, ins, out):
    nc = tc.nc
    scale = 1.0 / np.sqrt(D)

    xtb = ins["xtb"]    # [C, T] bf16 (x_b.T)
    wqa = ins["wqa"]    # [C, NH*D] bf16
    wka = ins["wka"]    # [C, NH*D] bf16
    wva = ins["wva"]    # [(CHX+1)*P, VW] bf16 (row C = bias / ones-col)
    wp = ins["wp"]      # [NH*D, C] bf16
    msk = ins["msk"]    # [P, 4*512] bf16 diagonal causal masks (row0 j0 = ones)
    bqk = ins["bqk"]    # [P, 4] f32 per-pair Q/K biases

    with ExitStack() as stk:
        pool = lambda name, bufs, space="SBUF": stk.enter_context(
            tc.tile_pool(name=name, bufs=bufs, space=space)
        )
        const = pool("const", 1)
        xpool = pool("xc", 2)
        qkv_sb = pool("qkv", 1)
        pt_pool = pool("pt", 26)
        stg_pool = pool("stg", 8)
        rc_pool = pool("rc", 8)
        ost_pool = pool("ost", 4)
        qk_ps = pool("qk_ps", 2, "PSUM")
        big_ps = pool("big_ps", 2, "PSUM")   # shared: V chains + projection
        st_ps = pool("st_ps", 2, "PSUM")
        ot_ps = pool("ot_ps", 2, "PSUM")

        # --- constant loads, all on the SP (sync) HWDGE queue, ordered by
        # first use: bqk+wqa before x chunk 0, wp (projection) last. ---
        bqk_sb = const.tile([P, 4], F32, tag="bqk")
        nc.sync.dma_start(bqk_sb[:], bqk[:, :])
        wqa_sb = const.tile([P, CHX, NH * D], BF16, tag="wqa")
        wqa_r = wqa.rearrange("(c p) n -> p c n", p=P)
        nc.sync.dma_start(wqa_sb[:, 0:4, :], wqa_r[:, 0:4, :])

        # --- persistent activations ---
        qt_all = qkv_sb.tile([P, NP, T], F32R, tag="qt")  # [pair 2x64, pair, T]
        kt_all = qkv_sb.tile([P, NP, T], F32R, tag="kt")
        v_all = qkv_sb.tile([P, NTC * 4, VW], BF16, tag="v")   # [t, tt, d|l]
        yt_all = qkv_sb.tile([P, NP, T], BF16, tag="yt")  # [pair 2x64, pair, T]

        xr = xtb.rearrange("(c p) t -> p c t", p=P)
        xc_tiles = {}

        def issue_xc(r):
            xcr = xpool.tile([P, CHX, 512], BF16, tag="xc")
            nc.sync.dma_start(xcr[:, 0:4, :], xr[:, 0:4, bass.ts(r, 512)])
            nc.sync.dma_start(xcr[:, 4:8, :], xr[:, 4:8, bass.ts(r, 512)])
            xc_tiles[r] = xcr

        nc.sync.dma_start(wqa_sb[:, 4:8, :], wqa_r[:, 4:8, :])
        issue_xc(0)
        wka_sb = const.tile([P, CHX, NH * D], BF16, tag="wka")
        nc.sync.dma_start(wka_sb[:], wka.rearrange("(c p) n -> p c n", p=P))
        wva_sb = const.tile([P, CHX + 1, VW], BF16, tag="wva")
        nc.sync.dma_start(wva_sb[:], wva.rearrange("(c p) n -> p c n", p=P))
        msk_sb = const.tile([P, 4, 512], BF16, tag="msk")
        nc.sync.dma_start(msk_sb[:], msk.rearrange("p (j f) -> p j f", j=4))
        wp_sb = const.tile([P, NP, C], BF16, tag="wp")
        nc.sync.dma_start(wp_sb[:], wp.rearrange("(c p) n -> p c n", p=P))

        def emit_qk(r, which, jj):
            """Q (which=0) or K (which=1) chain for pair jj, t-chunk r."""
            w_sb, dst = ((wqa_sb, qt_all), (wka_sb, kt_all))[which]
            xcr = xc_tiles[r]
            ps = qk_ps.tile([P, 512], F32, tag="qk")
            for c in range(CHX):
                nc.tensor.matmul(
                    ps[:], w_sb[:, c, bass.ts(jj, P)], xcr[:, c, :],
                    start=(c == 0), stop=(c == CHX - 1),
                )
            nc.vector.tensor_scalar_add(
                dst[:, jj, bass.ts(r, 512)], ps[:],
                bqk_sb[:, which * NP + jj : which * NP + jj + 1],
            )

        def emit_v(r, tt):
            """V rows for token tile r*4+tt, all heads + ones cols."""
            xcr = xc_tiles[r]
            ps = big_ps.tile([P, 512], F32, tag="big")
            pv = ps[:, 0:VW]
            for c in range(CHX):
                nc.tensor.matmul(
                    pv, xcr[:, c, bass.ts(tt, P)], wva_sb[:, c, :],
                    start=(c == 0), stop=False,
                )
            # rank-1 bias/ones row via 1-partition matmul (msk row0/j0 is ones)
            nc.tensor.matmul(
                pv, msk_sb[0:1, 0, 0:P], wva_sb[0:1, CHX, :],
                start=False, stop=True,
            )
            nc.vector.tensor_copy(v_all[:, r * 4 + tt, :], pv)

        cur_stg = {}
        live = []  # units with pending PV qt-groups (PSUM: one open group/bank)

        def finish_qt(u, qt):
            """Normalize y[:,qt] by 1/l into pair staging; transpose + (for the
            final slab) projection as soon as the pair's qt columns are done."""
            h, qc, ot = u["h"], u["qc"], u["ot"]
            hb, hj = (h % 2) * D, h // 2
            rc = rc_pool.tile([P, 1], F32, tag="rc")
            nc.vector.reciprocal(rc[:], ot[:, qt, D : D + 1])
            if h % 2 == 0:
                cur_stg[(hj, qt)] = stg_pool.tile(
                    [P, P], BF16, tag="stg", name="stg"
                )
            stg = cur_stg[(hj, qt)]
            nc.vector.tensor_scalar_mul(
                stg[:, hb : hb + D], ot[:, qt, 0:D], rc[:]
            )
            if h % 2 == 1:
                nc.sync.dma_start_transpose(
                    yt_all[:, hj, bass.ts(qc * 4 + qt, P)], stg[:]
                )
                if u.get("proj_after"):
                    emit_proj(qc, 2 * qt)
                    emit_proj(qc, 2 * qt + 1)

        def pump_pv():
            """Emit the oldest unit's next PV qt-group (contiguous in ot bank)."""
            if not live or not live[0]["pvgs"]:
                return
            u = live[0]
            qt = u["pvgs"].pop(0)
            h, qc = u["h"], u["qc"]
            last = qc * 4 + qt
            for kt in range(last + 1):
                nc.tensor.matmul(
                    u["ot"][:, qt, :],
                    u["pts"][kt][:, bass.ts(qt, P)],
                    v_all[:, kt, h * (D + 1) : (h + 1) * (D + 1)],
                    start=(kt == 0), stop=(kt == last),
                )
            finish_qt(u, qt)
            if not u["pvgs"]:
                live.pop(0)

        def emit_unit(h, qc):
            """S/exp/mask phase for unit (h, qc); PV of older units pumped in."""
            hb, hj = (h % 2) * D, h // 2
            nkt = 4 * (qc + 1)
            pts = []
            for kt in range(nkt):
                j = kt - (nkt - 4)
                lo = max(j, 0) * P
                st = st_ps.tile([P, 512], F32, tag="st")
                nc.tensor.matmul(
                    st[:, lo:512],
                    kt_all[hb : hb + D, hj, bass.ts(kt, P)],
                    qt_all[hb : hb + D, hj, qc * 512 + lo : (qc + 1) * 512],
                    start=True, stop=True,
                )
                pt = pt_pool.tile([P, 512], BF16, tag="pt")
                nc.scalar.activation(
                    pt[:, lo:512], st[:, lo:512],
                    mybir.ActivationFunctionType.Exp, scale=float(scale),
                )
                if j >= 0:
                    nc.vector.tensor_mul(
                        pt[:, lo:512], pt[:, lo:512], msk_sb[:, j, lo:512]
                    )
                pts.append(pt)
                pump_pv()
            ot = ot_ps.tile([P, 4, D + 1], F32, tag="ot")
            live.append({
                "h": h, "qc": qc, "ot": ot, "pts": pts, "pvgs": [0, 1, 2, 3],
                "proj_after": (h, qc) == (3, 3),
            })

        def emit_proj(pq, g):
            """Projection group g (tt=pq*4+g//2, cc=g%2) for slab pq."""
            tt, cc = pq * 4 + g // 2, g % 2
            ps = big_ps.tile([P, 512], F32, tag="big")
            for jj in range(NP):
                nc.tensor.matmul(
                    ps[:], yt_all[:, jj, bass.ts(tt, P)],
                    wp_sb[:, jj, bass.ts(cc, 512)],
                    start=(jj == 0), stop=(jj == NP - 1),
                )
            st = ost_pool.tile([P, 512], F32, tag="ost")
            nc.vector.tensor_copy(st[:], ps[:])
            nc.sync.dma_start(out[bass.ts(tt, P), bass.ts(cc, 512)], st[:])

        # ---------------- the fused pipeline ----------------
        for r in range(5):
            units = SCHED[r]
            if r == 0:
                emit_qk(0, 0, 0)
                emit_qk(0, 0, 1)
                emit_qk(0, 1, 0)
                emit_qk(0, 1, 1)
                issue_xc(1)
                for tt in range(4):
                    emit_v(0, tt)
                continue
            has_qkv = r <= 3
            if has_qkv:
                emit_qk(r, 0, 0)
                emit_qk(r, 0, 1)
                emit_qk(r, 1, 0)
                emit_qk(r, 1, 1)
                if r + 1 <= 3:
                    issue_xc(r + 1)
                emit_v(r, 0)
                emit_v(r, 1)
            pq = r - 2  # projection slab whose transposes landed last round
            n = len(units)
            for i, (h, qc) in enumerate(units):
                emit_unit(h, qc)
                if has_qkv and i == 0:
                    emit_v(r, 2)
                    emit_v(r, 3)
                if 0 <= pq < 3:
                    for g in range(8 * i // n, 8 * (i + 1) // n):
                        emit_proj(pq, g)
        while live:
            pump_pv()


def make_shard_inputs(x_b, w_attn, b_attn, w_proj, h0):
    """Per-core input dict for batch slice x_b [T, C] and heads h0..h0+NH-1."""
    bf = ml_dtypes.bfloat16
    xtb = np.ascontiguousarray(x_b.T).astype(bf)

    qs = slice(h0 * D, (h0 + NH) * D)
    ks = slice(C + h0 * D, C + (h0 + NH) * D)
    wqa = np.ascontiguousarray(w_attn[:, qs]).astype(bf)
    wka = np.ascontiguousarray(w_attn[:, ks]).astype(bf)

    wva = np.zeros((C + P, VW), dtype=np.float32)
    for h in range(NH):
        vs = slice(2 * C + (h0 + h) * D, 2 * C + (h0 + h + 1) * D)
        wva[:C, h * (D + 1) : h * (D + 1) + D] = w_attn[:, vs]
        wva[C, h * (D + 1) : h * (D + 1) + D] = b_attn[vs]
        wva[C, h * (D + 1) + D] = 1.0  # ones column -> softmax denominator
    wva = wva.astype(bf)

    wp = np.ascontiguousarray(w_proj[h0 * D : (h0 + NH) * D, :]).astype(bf)

    msk = np.zeros((P, 4 * 512), dtype=np.float32)
    p = np.arange(P)[:, None]
    f = np.arange(512)[None, :]
    for j in range(4):
        msk[:, j * 512 : (j + 1) * 512] = (j * P + p <= f).astype(np.float32)
    msk = msk.astype(bf)

    bqk = np.zeros((P, 4), dtype=np.float32)
    for j in range(NP):
        bqk[:, j] = b_attn[(h0 + 2 * j) * D : (h0 + 2 * j + 2) * D]
        bqk[:, NP + j] = b_attn[C + (h0 + 2 * j) * D : C + (h0 + 2 * j + 2) * D]

    return {
        "xtb": xtb, "wqa": wqa, "wka": wka, "wva": wva,
        "wp": wp, "msk": msk, "bqk": bqk,
    }


_NC_CACHE = {}


def _build_nc():
    if "nc" in _NC_CACHE:
        return _NC_CACHE["nc"]
    nc = bacc.Bacc("TRN2", target_bir_lowering=False, debug=False)
    in_specs = {
        "xtb": ((C, T), BF16),
        "wqa": ((C, NH * D), BF16),
        "wka": ((C, NH * D), BF16),
        "wva": ((C + P, VW), BF16),
        "wp": ((NH * D, C), BF16),
        "msk": ((P, 4 * 512), BF16),
        "bqk": ((P, 4), F32),
    }
    in_aps = {
        k: nc.dram_tensor(k, list(s), dt, kind="ExternalInput").ap()
        for k, (s, dt) in in_specs.items()
    }
    out_ap = nc.dram_tensor("out", [T, C], F32, kind="ExternalOutput").ap()
    with tile.TileContext(nc) as tc:
        build_tile_kernel(tc, in_aps, out_ap)
    nc.compile()
    _NC_CACHE["nc"] = nc
    return nc


def _run(inputs, trace=False):
    x = np.ascontiguousarray(inputs["x"], dtype=np.float32)
    w_attn = np.ascontiguousarray(inputs["w_attn"], dtype=np.float32)
    b_attn = np.ascontiguousarray(inputs["b_attn"], dtype=np.float32)
    w_proj = np.ascontiguousarray(inputs["w_proj"], dtype=np.float32)
    b_proj = np.ascontiguousarray(inputs["b_proj"], dtype=np.float32)

    nc = _build_nc()
    in_maps = [
        make_shard_inputs(x[c // 4], w_attn, b_attn, w_proj, (c % 4) * NH)
        for c in range(N_CORES)
    ]
    res = bass_utils.run_bass_kernel_spmd(
        nc, in_maps, core_ids=list(range(N_CORES)), trace=trace
    )
    out = np.zeros((B, T, C), dtype=np.float64)
    for c in range(N_CORES):
        out[c // 4] += res.results[c]["out"].astype(np.float64)
    out += b_proj.astype(np.float64)
    return out.astype(np.float32), res


def kernel(**inputs):
    out, _ = _run(inputs)
    return out
